# revision 1
# baseline (speedup 1.0000x reference)
"""Label-smoothing KLDiv loss (batchmean) on 8 Trainium2 NeuronCores.

Math: with fv = SMOOTHING/(V-K), lv = (1-SMOOTHING)/K, and per-row unique
label sets L_b (|L_b| = U_b), the reference loss decomposes exactly as

  loss * B = C - fv * S - (lv - fv) * G
  C = sum_b [ U_b*lv*ln(lv) + (V-U_b)*fv*ln(fv) ]     (host, closed form)
  S = sum_{b,v} output[b,v]                           (device, 412MB reduction)
  G = sum_b sum_{v in L_b} output[b,v]                (device, indirect gather)

Each core reduces a 256-row batch shard (51.5MB) with tensor-engine
ones-matmuls into PSUM, gathers its 1280 label logits via indirect DMA,
and returns [S_partial, G_partial]; the host combines in float64.

The shard is padded with 128 zeros: a global sum doesn't care how the flat
array splits across partitions, and duplicate labels in a row gather a
padded zero instead of needing a mask multiply on device.
"""

import math
from contextlib import ExitStack

import numpy as np

import concourse.bass as bass
import concourse.bass_isa as bass_isa
import concourse.mybir as mybir
from concourse.bass_utils import run_bass_kernel_spmd

B = 2048
V = 50257
K = 5
NCORES = 8
SMOOTHING = 0.1

RPC = B // NCORES          # rows per core: 256
NFLAT = RPC * V            # 12,865,792 data elems per core
PAD = 128
NTOT = NFLAT + PAD         # 12,865,920 = 128 * 100,515
P = 128
FPP = NTOT // P            # 100,515 elems per partition
F_TILE = 10240             # free-dim tile: 5MB DMAs, 40KB/partition
NBUF = 4                   # stream buffers (4 x 40KB = 160KB/partition)
MM_N = 512                 # fp32 moving-operand max per matmul
NG = (RPC * K) // P        # gather columns: 10

F32 = mybir.dt.float32
I32 = mybir.dt.int32

_CACHE: dict = {}


def build_module() -> bass.Bass:
    nc = bass.Bass()
    x = nc.dram_tensor("x", [NTOT], F32, kind="ExternalInput")
    gidx = nc.dram_tensor("gidx", [P, NG], I32, kind="ExternalInput")
    res = nc.dram_tensor("res", [P, 2], F32, kind="ExternalOutput")

    x_flat = x[:]
    x2d = x_flat.rearrange("(p f) -> p f", p=P)
    xcol = x_flat.rearrange("(n one) -> n one", one=1)  # [NTOT, 1] gather view

    n_full, rem = divmod(FPP, F_TILE)
    spans = [(t * F_TILE, F_TILE) for t in range(n_full)]
    if rem:
        spans.append((n_full * F_TILE, rem))
    ns = len(spans)

    # Raw-bass program: this toolchain's walrus rejects instructions with
    # more than one semaphore wait, so every instruction below is arranged
    # to carry at most one. A single DVE-progress sem (v_sem) sequences
    # slot recycling, the partition reduce, and the final store.
    with ExitStack() as ctx:
        xts = [
            ctx.enter_context(nc.sbuf_tensor(f"xt{i}", [P, F_TILE], F32))
            for i in range(NBUF)
        ]
        idx_sb = ctx.enter_context(nc.sbuf_tensor([P, NG], I32))
        g_sb = ctx.enter_context(nc.sbuf_tensor([P, NG], F32))
        acc = ctx.enter_context(nc.sbuf_tensor([P, ns + 1], F32))
        out_sb = ctx.enter_context(nc.sbuf_tensor([P, 2], F32))
        dma_sems = [
            ctx.enter_context(nc.semaphore(f"dma{i}")) for i in range(NBUF)
        ]
        o_sem = ctx.enter_context(nc.semaphore("o_sem"))
        gi_sem = ctx.enter_context(nc.semaphore("gi_sem"))
        gg_sem = ctx.enter_context(nc.semaphore("gg_sem"))
        v_sem = ctx.enter_context(nc.semaphore("v_sem"))
        block = ctx.enter_context(nc.Block())

        @block.sync
        def _(sync):
            # Stream the shard; recycle a slot once its reduce finished.
            for t, (off, fl) in enumerate(spans):
                if t >= NBUF:
                    sync.wait_ge(v_sem, t - NBUF + 1)
                sync.dma_start(
                    out=xts[t % NBUF][:, :fl], in_=x2d[:, off : off + fl]
                ).then_inc(dma_sems[t % NBUF], 16)
            sync.wait_ge(v_sem, ns + 2)
            sync.dma_start(out=res[:], in_=out_sb[:]).then_inc(o_sem, 16)

        @block.gpsimd
        def _(gpsimd):
            gpsimd.dma_start(out=idx_sb[:], in_=gidx[:]).then_inc(gi_sem, 16)
            gpsimd.wait_ge(gi_sem, 16)
            # Gather the 1280 label logits in one indirect DMA ([128,10]
            # offsets -> values); duplicate slots point at pad zeros.
            gpsimd.indirect_dma_start(
                out=g_sb[:, :],
                out_offset=None,
                in_=xcol,
                in_offset=bass.IndirectOffsetOnAxis(ap=idx_sb[:, :], axis=0),
            ).then_inc(gg_sem, 16)

        @block.vector
        def _(vector):
            for t, (off, fl) in enumerate(spans):
                vector.wait_ge(dma_sems[t % NBUF], 16 * (t // NBUF + 1))
                vector.reduce_sum(
                    out=acc[:, t : t + 1],
                    in_=xts[t % NBUF][:, :fl],
                    axis=mybir.AxisListType.X,
                ).then_inc(v_sem, 1)
            vector.wait_ge(gg_sem, 16)
            vector.reduce_sum(
                out=out_sb[:, 1:2],
                in_=g_sb[:, :],
                axis=mybir.AxisListType.X,
            ).then_inc(v_sem, 1)
            vector.wait_ge(v_sem, ns)  # all acc columns committed
            vector.reduce_sum(
                out=out_sb[:, 0:1],
                in_=acc[:, 0:ns],
                axis=mybir.AxisListType.X,
            ).then_inc(v_sem, 1)

    return nc


def get_nc() -> bass.Bass:
    if "nc" not in _CACHE:
        _CACHE["nc"] = build_module()
    return _CACHE["nc"]


def prepare_in_maps(output: np.ndarray, labels: np.ndarray):
    """Shard batch across cores; flat gather indices with duplicate labels
    redirected to the zero pad (so they count once, matching .at[].set)."""
    output = np.ascontiguousarray(np.asarray(output, dtype=np.float32))
    lab = np.asarray(labels).astype(np.int64)

    first = np.ones((B, K), dtype=bool)
    for k in range(1, K):
        first[:, k] = ~(lab[:, k : k + 1] == lab[:, :k]).any(axis=1)
    u_total = float(first.sum())

    pad = np.zeros(PAD, dtype=np.float32)
    in_maps = []
    for c in range(NCORES):
        rows = slice(c * RPC, (c + 1) * RPC)
        shard = np.concatenate([output[rows].reshape(-1), pad])
        local_b = np.arange(RPC, dtype=np.int64)[:, None]
        flat_idx = local_b * V + lab[rows]
        flat_idx[~first[rows]] = NFLAT  # first pad element == 0.0
        in_maps.append(
            {"x": shard, "gidx": flat_idx.reshape(P, NG).astype(np.int32)}
        )
    return in_maps, u_total


def combine(results, u_total: float) -> np.ndarray:
    s_total = sum(float(r["res"][:, 0].astype(np.float64).sum()) for r in results)
    g_total = sum(float(r["res"][:, 1].astype(np.float64).sum()) for r in results)
    fv = float(np.float32(SMOOTHING / (V - K)))
    lv = float(np.float32((1.0 - SMOOTHING) / K))
    c_term = u_total * lv * math.log(lv) + (B * V - u_total) * fv * math.log(fv)
    loss = (c_term - fv * s_total - (lv - fv) * g_total) / B
    return np.array(loss, dtype=np.float32)


def kernel(output: np.ndarray, labels: np.ndarray) -> np.ndarray:
    in_maps, u_total = prepare_in_maps(output, labels)
    results = run_bass_kernel_spmd(
        get_nc(), in_maps, core_ids=list(range(NCORES))
    ).results
    return combine(results, u_total)



# revision 2
# speedup vs baseline: 54146.1307x; 54146.1307x over previous
"""Label-smoothing KLDiv loss (batchmean) on 8 Trainium2 NeuronCores.

Math: with fv = SMOOTHING/(V-K), lv = (1-SMOOTHING)/K, and per-row unique
label sets L_b (|L_b| = U_b), the reference loss decomposes exactly as

  loss * B = C - fv * S - (lv - fv) * G
  C = sum_b [ U_b*lv*ln(lv) + (V-U_b)*fv*ln(fv) ]     (host, closed form)
  S = sum_{b,v} output[b,v]                           (device, 412MB reduction)
  G = sum_b sum_{v in L_b} output[b,v]                (device, indirect gather)

Each core streams a 256-row batch shard (51.5MB) through SBUF and reduces
it on the vector engine with scalar_tensor_tensor over tile PAIRS
(out=(a+0)+b with accum_out) — two SBUF reads per cycle, so the DVE runs
at 2 elem/cycle/lane and stays off the DMA critical path.  The last pair
is small (2x1106 columns) so almost no reduction work remains after the
final DMA lands.  The 1280 label logits are gathered with ten per-column
indirect DMAs (the indirect engine consumes ONE offset per partition and
copies a contiguous run, so each gathered element needs its own column).
The host combines partial S/G in float64 with the closed-form C.

The shard is padded with 256 zeros: a global sum doesn't care how the
flat array splits across partitions, and duplicate labels in a row gather
a padded zero instead of needing a mask multiply on device.
"""

import math
from contextlib import ExitStack

import numpy as np

import concourse.bass as bass
import concourse.mybir as mybir
from concourse.bass_utils import run_bass_kernel_spmd

B = 2048
V = 50257
K = 5
NCORES = 8
SMOOTHING = 0.1

RPC = B // NCORES          # rows per core: 256
NFLAT = RPC * V            # 12,865,792 data elems per core
PAD = 256
NTOT = NFLAT + PAD         # 12,866,048 = 128 * 100,516
P = 128
FPP = NTOT // P            # 100,516 elems per partition
F_BIG = 12288              # 48KB/partition per big tile
F_SMALL = 1106             # 8*F_BIG + 2*F_SMALL == FPP
SPANS = [F_BIG] * 8 + [F_SMALL] * 2
assert sum(SPANS) == FPP
NPAIR = len(SPANS) // 2    # 5 STT pair-reductions
NBUF = 4                   # 4 slots = 2 pairs in flight (192KB/partition)
NG = (RPC * K) // P        # gather columns: 10

F32 = mybir.dt.float32
I32 = mybir.dt.int32

_CACHE: dict = {}


def build_module() -> bass.Bass:
    nc = bass.Bass()
    x = nc.dram_tensor("x", [NTOT], F32, kind="ExternalInput")
    gidx = nc.dram_tensor("gidx", [P, NG], I32, kind="ExternalInput")
    res = nc.dram_tensor("res", [P, 2], F32, kind="ExternalOutput")

    x_flat = x[:]
    x2d = x_flat.rearrange("(p f) -> p f", p=P)
    xcol = x_flat.rearrange("(n one) -> n one", one=1)  # [NTOT, 1] gather view

    offs = [sum(SPANS[:t]) for t in range(len(SPANS))]
    add = mybir.AluOpType.add

    # Raw-bass program: this toolchain's walrus rejects instructions with
    # more than one semaphore wait, so every instruction below carries at
    # most one.  v_sem counts finished pair-reductions (slot recycling);
    # f_sem counts the two final reduces (store gate).
    with ExitStack() as ctx:
        xts = [
            ctx.enter_context(nc.sbuf_tensor(f"xt{i}", [P, F_BIG], F32))
            for i in range(NBUF)
        ]
        idx_sb = ctx.enter_context(nc.sbuf_tensor([P, NG], I32))
        g_sb = ctx.enter_context(nc.sbuf_tensor([P, NG], F32))
        acc = ctx.enter_context(nc.sbuf_tensor([P, NPAIR], F32))
        out_sb = ctx.enter_context(nc.sbuf_tensor([P, 2], F32))
        ps_sems = [
            ctx.enter_context(nc.semaphore(f"ps{i}")) for i in range(2)
        ]
        o_sem = ctx.enter_context(nc.semaphore("o_sem"))
        gi_sem = ctx.enter_context(nc.semaphore("gi_sem"))
        gg_sem = ctx.enter_context(nc.semaphore("gg_sem"))
        v_sem = ctx.enter_context(nc.semaphore("v_sem"))
        f_sem = ctx.enter_context(nc.semaphore("f_sem"))
        block = ctx.enter_context(nc.Block())

        @block.sync
        def _(sync):
            # Stream the shard; a slot pair recycles once its STT finished.
            for t, fl in enumerate(SPANS):
                k = t // 2
                if t >= NBUF:
                    sync.wait_ge(v_sem, k - 1)
                sync.dma_start(
                    out=xts[t % NBUF][:, :fl],
                    in_=x2d[:, offs[t] : offs[t] + fl],
                ).then_inc(ps_sems[k % 2], 16)
            sync.wait_ge(f_sem, 2)
            sync.dma_start(out=res[:], in_=out_sb[:]).then_inc(o_sem, 16)

        @block.gpsimd
        def _(gpsimd):
            gpsimd.dma_start(out=idx_sb[:], in_=gidx[:]).then_inc(gi_sem, 16)
            gpsimd.wait_ge(gi_sem, 16)
            # The indirect engine reads ONE offset per partition and copies
            # a contiguous run from it, so gather column-by-column: each
            # DMA fetches one scattered fp32 per partition.  Duplicate
            # labels point at pad zeros.
            for j in range(NG):
                gpsimd.indirect_dma_start(
                    out=g_sb[:, j : j + 1],
                    out_offset=None,
                    in_=xcol,
                    in_offset=bass.IndirectOffsetOnAxis(
                        ap=idx_sb[:, j : j + 1], axis=0
                    ),
                ).then_inc(gg_sem, 16)

        @block.vector
        def _(vector):
            # Pair k reads slots (2k%4, 2k%4+1); both arrivals bump the
            # same pair semaphore, so one wait covers both DMAs.
            for k in range(NPAIR):
                fl = SPANS[2 * k]
                sl = xts[(2 * k) % NBUF]
                sr = xts[(2 * k) % NBUF + 1]
                vector.wait_ge(ps_sems[k % 2], 32 * (k // 2 + 1))
                vector.scalar_tensor_tensor(
                    out=sl[:, :fl],
                    in0=sl[:, :fl],
                    scalar=0.0,
                    in1=sr[:, :fl],
                    op0=add,
                    op1=add,
                    accum_out=acc[:, k : k + 1],
                ).then_inc(v_sem, 1)
            vector.wait_ge(gg_sem, 16 * NG)
            vector.reduce_sum(
                out=out_sb[:, 1:2],
                in_=g_sb[:, :],
                axis=mybir.AxisListType.X,
            ).then_inc(f_sem, 1)
            vector.reduce_sum(
                out=out_sb[:, 0:1],
                in_=acc[:, :],
                axis=mybir.AxisListType.X,
            ).then_inc(f_sem, 1)

    return nc


def get_nc() -> bass.Bass:
    if "nc" not in _CACHE:
        _CACHE["nc"] = build_module()
    return _CACHE["nc"]


def prepare_in_maps(output: np.ndarray, labels: np.ndarray):
    """Shard batch across cores; flat gather indices with duplicate labels
    redirected to the zero pad (so they count once, matching .at[].set)."""
    output = np.ascontiguousarray(np.asarray(output, dtype=np.float32))
    lab = np.asarray(labels).astype(np.int64)

    first = np.ones((B, K), dtype=bool)
    for k in range(1, K):
        first[:, k] = ~(lab[:, k : k + 1] == lab[:, :k]).any(axis=1)
    u_total = float(first.sum())

    pad = np.zeros(PAD, dtype=np.float32)
    in_maps = []
    for c in range(NCORES):
        rows = slice(c * RPC, (c + 1) * RPC)
        shard = np.concatenate([output[rows].reshape(-1), pad])
        local_b = np.arange(RPC, dtype=np.int64)[:, None]
        flat_idx = local_b * V + lab[rows]
        flat_idx[~first[rows]] = NFLAT  # first pad element == 0.0
        in_maps.append(
            {"x": shard, "gidx": flat_idx.reshape(P, NG).astype(np.int32)}
        )
    return in_maps, u_total


def combine(results, u_total: float) -> np.ndarray:
    s_total = sum(float(r["res"][:, 0].astype(np.float64).sum()) for r in results)
    g_total = sum(float(r["res"][:, 1].astype(np.float64).sum()) for r in results)
    fv = float(np.float32(SMOOTHING / (V - K)))
    lv = float(np.float32((1.0 - SMOOTHING) / K))
    c_term = u_total * lv * math.log(lv) + (B * V - u_total) * fv * math.log(fv)
    loss = (c_term - fv * s_total - (lv - fv) * g_total) / B
    return np.array(loss, dtype=np.float32)


def kernel(output: np.ndarray, labels: np.ndarray) -> np.ndarray:
    in_maps, u_total = prepare_in_maps(output, labels)
    results = run_bass_kernel_spmd(
        get_nc(), in_maps, core_ids=list(range(NCORES))
    ).results
    return combine(results, u_total)


# revision 4
# speedup vs baseline: 56786.9741x; 1.0488x over previous
"""Label-smoothing KLDiv loss (batchmean) on 8 Trainium2 NeuronCores.

Math: with fv = SMOOTHING/(V-K), lv = (1-SMOOTHING)/K, and per-row unique
label sets L_b (|L_b| = U_b), the reference loss decomposes exactly as

  loss * B = C - fv * S - (lv - fv) * G
  C = sum_b [ U_b*lv*ln(lv) + (V-U_b)*fv*ln(fv) ]     (host, closed form)
  S = sum_{b,v} output[b,v]                           (device, 412MB reduction)
  G = sum_b sum_{v in L_b} output[b,v]                (device, indirect gather)

Each core streams a 256-row batch shard (51.5MB) through SBUF and reduces
it on the vector engine with scalar_tensor_tensor over tile PAIRS
(out=(a+0)+b with accum_out) — two SBUF reads per cycle, so the DVE runs
at 2 elem/cycle/lane and stays off the DMA critical path.  Tiles are
small enough (and the last pair smaller still) that the trailing
reduction after the final DMA lands is only a few microseconds.  The 1280 label logits are gathered with ten per-column
indirect DMAs (the indirect engine consumes ONE offset per partition and
copies a contiguous run, so each gathered element needs its own column).
The host combines partial S/G in float64 with the closed-form C.

The shard is padded with 256 zeros: a global sum doesn't care how the
flat array splits across partitions, and duplicate labels in a row gather
a padded zero instead of needing a mask multiply on device.
"""

import math
from contextlib import ExitStack

import numpy as np

import concourse.bass as bass
import concourse.mybir as mybir
from concourse.bass_utils import run_bass_kernel_spmd

B = 2048
V = 50257
K = 5
NCORES = 8
SMOOTHING = 0.1

RPC = B // NCORES          # rows per core: 256
NFLAT = RPC * V            # 12,865,792 data elems per core
PAD = 256
NTOT = NFLAT + PAD         # 12,866,048 = 128 * 100,516
P = 128
FPP = NTOT // P            # 100,516 elems per partition
F_BIG = 6900               # 27.6KB/partition per big tile
F_SMALL = 1958             # 14*F_BIG + 2*F_SMALL == FPP
SPANS = [F_BIG] * 14 + [F_SMALL] * 2
assert sum(SPANS) == FPP
NPAIR = len(SPANS) // 2    # 8 STT pair-reductions
NBUF = 4                   # 4 slots = 2 pairs in flight (110KB/partition)
NG = (RPC * K) // P        # gather columns: 10

F32 = mybir.dt.float32
I32 = mybir.dt.int32

_CACHE: dict = {}


def build_module() -> bass.Bass:
    nc = bass.Bass()
    x = nc.dram_tensor("x", [NTOT], F32, kind="ExternalInput")
    gidx = nc.dram_tensor("gidx", [P, NG], I32, kind="ExternalInput")
    res = nc.dram_tensor("res", [P, 2], F32, kind="ExternalOutput")

    x_flat = x[:]
    x2d = x_flat.rearrange("(p f) -> p f", p=P)
    xcol = x_flat.rearrange("(n one) -> n one", one=1)  # [NTOT, 1] gather view

    offs = [sum(SPANS[:t]) for t in range(len(SPANS))]
    add = mybir.AluOpType.add

    # Raw-bass program: this toolchain's walrus rejects instructions with
    # more than one semaphore wait, so every instruction below carries at
    # most one.  v_sem counts finished pair-reductions (slot recycling);
    # f_sem counts the two final reduces (store gate).
    with ExitStack() as ctx:
        xts = [
            ctx.enter_context(nc.sbuf_tensor(f"xt{i}", [P, F_BIG], F32))
            for i in range(NBUF)
        ]
        idx_sb = ctx.enter_context(nc.sbuf_tensor([P, NG], I32))
        g_sb = ctx.enter_context(nc.sbuf_tensor([P, NG], F32))
        acc = ctx.enter_context(nc.sbuf_tensor([P, NPAIR], F32))
        out_sb = ctx.enter_context(nc.sbuf_tensor([P, 2], F32))
        ps_sems = [
            ctx.enter_context(nc.semaphore(f"ps{i}")) for i in range(2)
        ]
        o_sem = ctx.enter_context(nc.semaphore("o_sem"))
        gi_sem = ctx.enter_context(nc.semaphore("gi_sem"))
        gg_sem = ctx.enter_context(nc.semaphore("gg_sem"))
        v_sem = ctx.enter_context(nc.semaphore("v_sem"))
        f_sem = ctx.enter_context(nc.semaphore("f_sem"))
        block = ctx.enter_context(nc.Block())

        @block.sync
        def _(sync):
            # Stream the shard; a slot pair recycles once its STT finished.
            for t, fl in enumerate(SPANS):
                k = t // 2
                if t >= NBUF:
                    sync.wait_ge(v_sem, k - 1)
                sync.dma_start(
                    out=xts[t % NBUF][:, :fl],
                    in_=x2d[:, offs[t] : offs[t] + fl],
                ).then_inc(ps_sems[k % 2], 16)
            sync.wait_ge(f_sem, 2)
            sync.dma_start(out=res[:], in_=out_sb[:]).then_inc(o_sem, 16)

        @block.gpsimd
        def _(gpsimd):
            gpsimd.dma_start(out=idx_sb[:], in_=gidx[:]).then_inc(gi_sem, 16)
            gpsimd.wait_ge(gi_sem, 16)
            # The indirect engine reads ONE offset per partition and copies
            # a contiguous run from it, so gather column-by-column: each
            # DMA fetches one scattered fp32 per partition.  Duplicate
            # labels point at pad zeros.
            for j in range(NG):
                gpsimd.indirect_dma_start(
                    out=g_sb[:, j : j + 1],
                    out_offset=None,
                    in_=xcol,
                    in_offset=bass.IndirectOffsetOnAxis(
                        ap=idx_sb[:, j : j + 1], axis=0
                    ),
                ).then_inc(gg_sem, 16)

        @block.vector
        def _(vector):
            # Pair k reads slots (2k%4, 2k%4+1); both arrivals bump the
            # same pair semaphore, so one wait covers both DMAs.
            for k in range(NPAIR):
                fl = SPANS[2 * k]
                sl = xts[(2 * k) % NBUF]
                sr = xts[(2 * k) % NBUF + 1]
                vector.wait_ge(ps_sems[k % 2], 32 * (k // 2 + 1))
                vector.scalar_tensor_tensor(
                    out=sl[:, :fl],
                    in0=sl[:, :fl],
                    scalar=0.0,
                    in1=sr[:, :fl],
                    op0=add,
                    op1=add,
                    accum_out=acc[:, k : k + 1],
                ).then_inc(v_sem, 1)
            vector.wait_ge(gg_sem, 16 * NG)
            vector.reduce_sum(
                out=out_sb[:, 1:2],
                in_=g_sb[:, :],
                axis=mybir.AxisListType.X,
            ).then_inc(f_sem, 1)
            vector.reduce_sum(
                out=out_sb[:, 0:1],
                in_=acc[:, :],
                axis=mybir.AxisListType.X,
            ).then_inc(f_sem, 1)

    return nc


def get_nc() -> bass.Bass:
    if "nc" not in _CACHE:
        _CACHE["nc"] = build_module()
    return _CACHE["nc"]


def prepare_in_maps(output: np.ndarray, labels: np.ndarray):
    """Shard batch across cores; flat gather indices with duplicate labels
    redirected to the zero pad (so they count once, matching .at[].set)."""
    output = np.ascontiguousarray(np.asarray(output, dtype=np.float32))
    lab = np.asarray(labels).astype(np.int64)

    first = np.ones((B, K), dtype=bool)
    for k in range(1, K):
        first[:, k] = ~(lab[:, k : k + 1] == lab[:, :k]).any(axis=1)
    u_total = float(first.sum())

    pad = np.zeros(PAD, dtype=np.float32)
    in_maps = []
    for c in range(NCORES):
        rows = slice(c * RPC, (c + 1) * RPC)
        shard = np.concatenate([output[rows].reshape(-1), pad])
        local_b = np.arange(RPC, dtype=np.int64)[:, None]
        flat_idx = local_b * V + lab[rows]
        flat_idx[~first[rows]] = NFLAT  # first pad element == 0.0
        in_maps.append(
            {"x": shard, "gidx": flat_idx.reshape(P, NG).astype(np.int32)}
        )
    return in_maps, u_total


def combine(results, u_total: float) -> np.ndarray:
    s_total = sum(float(r["res"][:, 0].astype(np.float64).sum()) for r in results)
    g_total = sum(float(r["res"][:, 1].astype(np.float64).sum()) for r in results)
    fv = float(np.float32(SMOOTHING / (V - K)))
    lv = float(np.float32((1.0 - SMOOTHING) / K))
    c_term = u_total * lv * math.log(lv) + (B * V - u_total) * fv * math.log(fv)
    loss = (c_term - fv * s_total - (lv - fv) * g_total) / B
    return np.array(loss, dtype=np.float32)


def kernel(output: np.ndarray, labels: np.ndarray) -> np.ndarray:
    in_maps, u_total = prepare_in_maps(output, labels)
    results = run_bass_kernel_spmd(
        get_nc(), in_maps, core_ids=list(range(NCORES))
    ).results
    return combine(results, u_total)
